# revision 9
# baseline (speedup 1.0000x reference)
"""BSplineKan layer kernel for 8 trn2 NeuronCores (steady-state opt).

Math: out[b,o] = w_b*sum_i silu(x[b,i]) + w_s*sum_{i,k} bases_k(x[b,i]) * P[o,i,k]
reformulated as 9 truncated-power feature plane matmuls per input element plus
a host-side bias (see fold_weights). The silu term depends only on x, so the
host computes it in numpy; the device does only the 9-plane contraction.

The kernel is POWER-bound, not schedule-bound: the PE clock is throttled by
data toggle rate (measured: same NEFF runs 62us/rep on zero data, 100+us on
random data). Design choices that matter are energy choices:
  - fp16 matmul operands (halves SBUF/xbus streaming energy; fp32r ldw-opt
    is unnecessary since FWL is compiler-automatic for fp16). Rel err 7e-4.
  - WEIGHTS ARE THE STATIONARY OPERAND, features the moving one: the moving
    stream is 4x the elements of the stationary load (512x128 vs 128x128 per
    MM), and the feature planes are ~59% exact zeros with small magnitudes,
    so streaming features instead of dense-random weights minimizes toggle
    power. Output comes out transposed (yT[o, b]); the host untransposes.
  - Feature production runs ONE CHUNK AHEAD of the matmuls (4 chunks of 512
    batch rows, double-buffered ft), wrapping across the For_i barrier so
    the post-barrier PE head is zero. No warm-up matmuls.
  - Per chunk: 8 psum banks, one per o-tile, 9-plane accumulation chains;
    bank drains (4 DVE + 4 ACT) into one fp16 tile, ONE coalesced 1MB y DMA
    per chunk. r intermediates are fp16 to halve the gate->square traffic.
  - Only the Square activation table is used (silu on host): no act-table
    switches in the loop.

Sharding: contraction split — core c owns i in [128c, 128c+128). Host sums
the 8 fp16 partials in fp64 and adds bias + w_b * silu.
"""

import numpy as np

import concourse.bass as bass
import concourse.bass_utils as _bu
import concourse.mybir as mybir
import concourse.tile as tile
from concourse import bacc
from concourse.bass_utils import run_bass_kernel_spmd

F32 = mybir.dt.float32
F16 = mybir.dt.float16
AF = mybir.ActivationFunctionType
ALU = mybir.AluOpType

B, I, O = 2048, 1024, 1024
N_CORES = 8
I_LOC = I // N_CORES       # 128 contraction rows per core
H = 2.25 / 15.0            # knot spacing 0.15
KNOTS = [j * H - 1.125 for j in range(8, 15)]   # interior knots in (0,1)
LEFT = KNOTS[:3]           # relu^2(c - x) knots
RIGHT = KNOTS[3:]          # relu^2(x - c) knots
N_PLANES = 9               # v, v^2, 3 left, 4 right
N_OT = O // 128            # 8 output tiles (one psum bank each)
NCH = 4                    # batch chunks per rep
BC = B // NCH              # 512 rows per chunk

# kept for test.py compatibility (fp16 needs no walrus ldw-opt flag).
_orig_run_command = _bu.run_command


def _run_command_ldwopt(argv, **kwargs):
    return _orig_run_command(argv, **kwargs)


def fold_weights(P: np.ndarray, w_s: float):
    """Fold spline parameters into per-plane weights.

    Returns W (N_PLANES, I, O) float16 and bias (O,) float64.
    """
    Pd = P.astype(np.float64)
    O_, I_, _ = P.shape
    Pz = np.zeros((O_, I_, 18))
    Pz[:, :, 5:13] = Pd[:, :, 5:13]
    G = np.zeros((O_, I_, 15))
    for j in range(5, 15):
        G[:, :, j] = (0.5 * Pz[:, :, j] - 1.5 * Pz[:, :, j - 1]
                      + 1.5 * Pz[:, :, j - 2] - 0.5 * Pz[:, :, j - 3])
    c = np.array([j * H - 1.125 for j in range(15)])
    inv_h2 = 1.0 / (H * H)
    A = (G[:, :, 5] + G[:, :, 6] + G[:, :, 7]) * inv_h2
    Bq = -2.0 * (c[5] * G[:, :, 5] + c[6] * G[:, :, 6] + c[7] * G[:, :, 7]) * inv_h2
    Cq = (c[5] ** 2 * G[:, :, 5] + c[6] ** 2 * G[:, :, 6] + c[7] ** 2 * G[:, :, 7]) * inv_h2
    D = [G[:, :, 8 + t] * inv_h2 for t in range(7)]
    left_w = []
    for t, cj in enumerate(LEFT):
        A += D[t]
        Bq += -2.0 * cj * D[t]
        Cq += cj * cj * D[t]
        left_w.append(-D[t])
    right_w = [D[3 + t] for t in range(4)]
    planes = [Bq + A, A] + left_w + right_w                   # each (O, I)
    bias = (Cq + 0.5 * Bq + 0.25 * A).sum(axis=1) * w_s       # (O,)
    W = np.empty((N_PLANES, I_, O_), np.float16)
    for p, pw in enumerate(planes):
        W[p] = (w_s * pw).T.astype(np.float16)
    return W, bias


def build_kernel(reps: int = 1, unroll: int = 1, loop_unroll: int = 4):
    """Per-core Bass kernel (SPMD across 8 cores, contraction-split).

    reps>1 wraps the body in a hardware loop, emitting the body
    loop_unroll times per iteration. unroll>1 emits the body N times with
    no loop (sim-only steady-state measurement).
    """
    nc = bacc.Bacc("TRN2", target_bir_lowering=False, debug=False,
                   num_devices=N_CORES)
    xT_d = nc.dram_tensor("xT", [I_LOC, B], F32, kind="ExternalInput")
    W_d = nc.dram_tensor("Wf", [N_PLANES * I_LOC, O], F16, kind="ExternalInput")
    yT_d = nc.dram_tensor("yT", [O, B], F16, kind="ExternalOutput")

    with tile.TileContext(nc) as tc:
        with (
            tc.tile_pool(name="wp", bufs=1) as w_pool,
            tc.tile_pool(name="xp", bufs=2) as x_pool,
            tc.tile_pool(name="fp", bufs=2) as f_pool,
            tc.tile_pool(name="sp", bufs=2) as s_pool,
            tc.tile_pool(name="op", bufs=2) as o_pool,
            tc.tile_pool(name="cp", bufs=1) as c_pool,
            tc.tile_pool(name="ps", bufs=1, space="PSUM") as ps_pool,
        ):
            consts = c_pool.tile([128, 1], F32, name="consts")
            nc.vector.memset(consts[:, 0:1], -0.5)

            # ---- hoisted: weights resident in SBUF across all reps ----
            wt = w_pool.tile([128, N_PLANES * O], F16, name="wt")
            wsrc = W_d[:].rearrange("(j p) o -> p j o", p=128)
            w3 = wt[:].rearrange("p (j o) -> p j o", j=N_PLANES)
            for j in range(N_PLANES):
                eng = nc.sync if j % 2 == 0 else nc.scalar
                eng.dma_start(w3[:, j, :], wsrc[:, j, :])

            state = {}

            def load_xt():
                """Issue the DMA for the NEXT body's xT slice."""
                xt = x_pool.tile([128, B], F32, tag="xt", name="xt")
                nc.sync.dma_start(xt[:, 0:BC], xT_d[:, 0:BC])
                nc.scalar.dma_start(xt[:, BC:], xT_d[:, BC:])
                return xt

            def produce(ft, xs, ch):
                """Write the 9 feature planes for batch slice xs into ft.

                Only the first op reads the fp32 x slice; everything else
                reads the fp16 v plane (16-bit DVE mode, half the traffic).
                Equivalent to evaluating the spline at fp16-rounded x-0.5;
                the f'*dx error is ~4e-4 relative, inside budget.
                """
                def plane(p):
                    return ft[:, p * BC:(p + 1) * BC]

                v = plane(0)
                nc.vector.tensor_scalar(v, xs, 0.5, None, ALU.subtract)
                nc.scalar.activation(plane(1), v, AF.Square)
                for t, cj in enumerate(LEFT + RIGHT):
                    gate = ALU.min if t < 3 else ALU.max
                    r = s_pool.tile([128, BC], F16, tag="r", name=f"r{ch}_{t}")
                    nc.vector.tensor_scalar(r[:], v, float(cj) - 0.5, 0.0,
                                            ALU.subtract, gate)
                    nc.scalar.activation(plane(2 + t), r[:], AF.Square)

            def mm_chunk(ft, ch):
                """8 o-tile psum banks, 9-plane accumulation; features move."""
                otc = o_pool.tile([128, N_OT * BC], F16, tag="otc",
                                  name=f"otc{ch}")
                for ot in range(N_OT):
                    ps = ps_pool.tile([128, BC], F32, tag=f"ps{ot}",
                                      name=f"ps{ch}_{ot}")
                    for j in range(N_PLANES):
                        nc.tensor.matmul(
                            ps[:],
                            wt[:, j * O + ot * 128:j * O + (ot + 1) * 128],
                            ft[:, j * BC:(j + 1) * BC],
                            start=(j == 0), stop=(j == N_PLANES - 1),
                        )
                    dst = otc[:, ot * BC:(ot + 1) * BC]
                    if ot % 2 == 0:
                        nc.vector.tensor_copy(dst, ps[:])
                    else:
                        nc.scalar.copy(dst, ps[:])
                eng = nc.sync if ch % 2 == 0 else nc.scalar
                eng.dma_start(
                    yT_d[:, ch * BC:(ch + 1) * BC].rearrange(
                        "(t p) b -> p t b", p=128),
                    otc[:].rearrange("p (t b) -> p t b", t=N_OT))

            def body(_iv=None):
                xt = state['xt']
                nxt = load_xt()          # next body's x, DMA overlaps this body
                for ch in range(NCH):
                    ft_cur = state['ft']
                    # produce the NEXT chunk's features (chunk 0 of the next
                    # body when ch==3) while this chunk's matmuls run
                    ft_next = f_pool.tile([128, N_PLANES * BC], F16,
                                          tag="ft", name=f"ft{(ch + 1) % 2}")
                    if ch < NCH - 1:
                        produce(ft_next, xt[:, (ch + 1) * BC:(ch + 2) * BC], ch + 1)
                    else:
                        produce(ft_next, nxt[:, 0:BC], 0)
                    mm_chunk(ft_cur, ch)
                    state['ft'] = ft_next
                state['xt'] = nxt

            def head():
                xt0 = x_pool.tile([128, B], F32, tag="xt", name="xt_h")
                nc.sync.dma_start(xt0[:, 0:BC], xT_d[:, 0:BC])
                nc.scalar.dma_start(xt0[:, BC:], xT_d[:, BC:])
                ft0 = f_pool.tile([128, N_PLANES * BC], F16, tag="ft",
                                  name="ft_h")
                produce(ft0, xt0[:, 0:BC], 0)
                state.update(xt=xt0, ft=ft0)

            head()
            if unroll > 1:
                assert reps == 1
                for _ in range(unroll):
                    body()
            elif reps == 1:
                body()
            else:
                assert reps % loop_unroll == 0
                with tc.For_i(0, reps // loop_unroll, 1) as iv:
                    for _ in range(loop_unroll):
                        body(iv)
    nc.compile()
    return nc


_cached_nc = None


def _get_nc():
    global _cached_nc
    if _cached_nc is None:
        _cached_nc = build_kernel(reps=1)
    return _cached_nc


def prepare_inputs(x, spline_parameters, w_b, w_s):
    """Host-side prep: returns (in_maps, bias, w_b, silu_sum) for the 8 cores."""
    x = np.ascontiguousarray(np.asarray(x, np.float32))
    P = np.asarray(spline_parameters, np.float32)
    w_b = float(np.asarray(w_b))
    W, bias = fold_weights(P, float(np.asarray(w_s)))
    xd = x.astype(np.float64)
    silu_sum = (xd / (1.0 + np.exp(-xd))).sum(axis=1)          # (B,)
    xT = np.ascontiguousarray(x.T)                             # (I, B)
    in_maps = []
    for c in range(N_CORES):
        sl = slice(c * I_LOC, (c + 1) * I_LOC)
        in_maps.append({
            "xT": np.ascontiguousarray(xT[sl, :]),
            "Wf": np.ascontiguousarray(
                W[:, sl, :].reshape(N_PLANES * I_LOC, O)),
        })
    return in_maps, bias, w_b, silu_sum


def kernel(x, spline_parameters, w_b, w_s):
    in_maps, bias, w_b, silu_sum = prepare_inputs(x, spline_parameters, w_b, w_s)
    nc = _get_nc()
    try:
        res = run_bass_kernel_spmd(nc, in_maps, core_ids=list(range(N_CORES)))
    except Exception:
        # transient device flakes (NRT_EXEC_UNIT_UNRECOVERABLE) happen; retry
        res = run_bass_kernel_spmd(nc, in_maps, core_ids=list(range(N_CORES)))
    acc = np.zeros((O, B), np.float64)
    for c in range(N_CORES):
        acc += res.results[c]["yT"].astype(np.float64)
    out = acc.T + bias[None, :]
    out += (w_b * silu_sum)[:, None]
    return out.astype(np.float32)


# revision 11
# speedup vs baseline: 1.0285x; 1.0285x over previous
"""BSplineKan layer kernel for 8 trn2 NeuronCores (steady-state opt).

Math: out[b,o] = w_b*sum_i silu(x[b,i]) + w_s*sum_{i,k} bases_k(x[b,i]) * P[o,i,k]
reformulated as 9 truncated-power feature plane matmuls per input element plus
a host-side bias (see fold_weights). The silu term depends only on x, so the
host computes it in numpy; the device does only the 9-plane contraction.

The kernel is POWER-bound, not schedule-bound: the PE clock is throttled by
data toggle rate (measured: same NEFF runs 62us/rep on zero data, 100+us on
random data). Design choices that matter are energy choices:
  - fp16 matmul operands (halves SBUF/xbus streaming energy; fp32r ldw-opt
    is unnecessary since FWL is compiler-automatic for fp16). Rel err 7e-4.
  - WEIGHTS ARE THE STATIONARY OPERAND, features the moving one: the moving
    stream is 4x the elements of the stationary load (512x128 vs 128x128 per
    MM), and the feature planes are ~59% exact zeros with small magnitudes,
    so streaming features instead of dense-random weights minimizes toggle
    power. Output comes out transposed (yT[o, b]); the host untransposes.
  - Feature production runs ONE CHUNK AHEAD of the matmuls (4 chunks of 512
    batch rows, double-buffered ft), wrapping across the For_i barrier so
    the post-barrier PE head is zero. No warm-up matmuls.
  - Per chunk: 8 psum banks, one per o-tile, 9-plane accumulation chains;
    bank drains (4 DVE + 4 ACT) into one fp16 tile, ONE coalesced 1MB y DMA
    per chunk. r intermediates are fp16 to halve the gate->square traffic.
  - Only the Square activation table is used (silu on host): no act-table
    switches in the loop.

Sharding: contraction split — core c owns i in [128c, 128c+128). Host sums
the 8 fp16 partials in fp64 and adds bias + w_b * silu.
"""

import numpy as np

import concourse.bass as bass
import concourse.bass_utils as _bu
import concourse.mybir as mybir
import concourse.tile as tile
from concourse import bacc
from concourse.bass_utils import run_bass_kernel_spmd

F32 = mybir.dt.float32
F16 = mybir.dt.float16
AF = mybir.ActivationFunctionType
ALU = mybir.AluOpType

B, I, O = 2048, 1024, 1024
N_CORES = 8
I_LOC = I // N_CORES       # 128 contraction rows per core
H = 2.25 / 15.0            # knot spacing 0.15
KNOTS = [j * H - 1.125 for j in range(8, 15)]   # interior knots in (0,1)
LEFT = KNOTS[:3]           # relu^2(c - x) knots
RIGHT = KNOTS[3:]          # relu^2(x - c) knots
N_PLANES = 9               # v, v^2, 3 left, 4 right
N_OT = O // 128            # 8 output tiles (one psum bank each)
NCH = 4                    # batch chunks per rep
BC = B // NCH              # 512 rows per chunk

# kept for test.py compatibility (fp16 needs no walrus ldw-opt flag).
_orig_run_command = _bu.run_command


def _run_command_ldwopt(argv, **kwargs):
    return _orig_run_command(argv, **kwargs)


def fold_weights(P: np.ndarray, w_s: float):
    """Fold spline parameters into per-plane weights.

    Returns W (N_PLANES, I, O) float16 and bias (O,) float64.
    """
    Pd = P.astype(np.float64)
    O_, I_, _ = P.shape
    Pz = np.zeros((O_, I_, 18))
    Pz[:, :, 5:13] = Pd[:, :, 5:13]
    G = np.zeros((O_, I_, 15))
    for j in range(5, 15):
        G[:, :, j] = (0.5 * Pz[:, :, j] - 1.5 * Pz[:, :, j - 1]
                      + 1.5 * Pz[:, :, j - 2] - 0.5 * Pz[:, :, j - 3])
    c = np.array([j * H - 1.125 for j in range(15)])
    inv_h2 = 1.0 / (H * H)
    A = (G[:, :, 5] + G[:, :, 6] + G[:, :, 7]) * inv_h2
    Bq = -2.0 * (c[5] * G[:, :, 5] + c[6] * G[:, :, 6] + c[7] * G[:, :, 7]) * inv_h2
    Cq = (c[5] ** 2 * G[:, :, 5] + c[6] ** 2 * G[:, :, 6] + c[7] ** 2 * G[:, :, 7]) * inv_h2
    D = [G[:, :, 8 + t] * inv_h2 for t in range(7)]
    left_w = []
    for t, cj in enumerate(LEFT):
        A += D[t]
        Bq += -2.0 * cj * D[t]
        Cq += cj * cj * D[t]
        left_w.append(-D[t])
    right_w = [D[3 + t] for t in range(4)]
    planes = [Bq + A, A] + left_w + right_w                   # each (O, I)
    bias = (Cq + 0.5 * Bq + 0.25 * A).sum(axis=1) * w_s       # (O,)
    W = np.empty((N_PLANES, I_, O_), np.float16)
    for p, pw in enumerate(planes):
        W[p] = (w_s * pw).T.astype(np.float16)
    return W, bias


def build_kernel(reps: int = 1, unroll: int = 1, loop_unroll: int = 4,
                 hint: bool = False, staggered: bool = True):
    """Per-core Bass kernel (SPMD across 8 cores, contraction-split).

    reps>1 wraps the body in a hardware loop, emitting the body
    loop_unroll times per iteration. unroll>1 emits the body N times with
    no loop (sim-only steady-state measurement).
    """
    nc = bacc.Bacc("TRN2", target_bir_lowering=False, debug=False,
                   num_devices=N_CORES)
    xT_d = nc.dram_tensor("xT", [I_LOC, B], F32, kind="ExternalInput")
    W_d = nc.dram_tensor("Wf", [N_PLANES * I_LOC, O], F16, kind="ExternalInput")
    yT_d = nc.dram_tensor("yT", [O, B], F16, kind="ExternalOutput")

    with tile.TileContext(nc) as tc:
        with (
            tc.tile_pool(name="wp", bufs=1) as w_pool,
            tc.tile_pool(name="xp", bufs=2) as x_pool,
            tc.tile_pool(name="fp", bufs=2) as f_pool,
            tc.tile_pool(name="sp", bufs=2) as s_pool,
            tc.tile_pool(name="op", bufs=2) as o_pool,
            tc.tile_pool(name="cp", bufs=1) as c_pool,
            tc.tile_pool(name="ps", bufs=1, space="PSUM") as ps_pool,
        ):
            consts = c_pool.tile([128, 1], F32, name="consts")
            nc.vector.memset(consts[:, 0:1], -0.5)

            # ---- hoisted: weights resident in SBUF across all reps ----
            wt = w_pool.tile([128, N_PLANES * O], F16, name="wt")
            wsrc = W_d[:].rearrange("(j p) o -> p j o", p=128)
            w3 = wt[:].rearrange("p (j o) -> p j o", j=N_PLANES)
            for j in range(N_PLANES):
                eng = nc.sync if j % 2 == 0 else nc.scalar
                eng.dma_start(w3[:, j, :], wsrc[:, j, :])

            state = {}

            def load_xt():
                """Issue the DMA for the NEXT body's xT slice."""
                xt = x_pool.tile([128, B], F32, tag="xt", name="xt")
                nc.sync.dma_start(xt[:, 0:BC], xT_d[:, 0:BC])
                nc.scalar.dma_start(xt[:, BC:], xT_d[:, BC:])
                return xt

            def produce(ft, xs, ch):
                """Write the 9 feature planes for batch slice xs into ft.

                Only the first op reads the fp32 x slice; everything else
                reads the fp16 v plane (16-bit DVE mode, half the traffic).
                Equivalent to evaluating the spline at fp16-rounded x-0.5;
                the f'*dx error is ~4e-4 relative, inside budget.
                """
                def plane(p):
                    return ft[:, p * BC:(p + 1) * BC]

                v = plane(0)
                nc.vector.tensor_scalar(v, xs, 0.5, None, ALU.subtract)
                nc.scalar.activation(plane(1), v, AF.Square)
                for t, cj in enumerate(LEFT + RIGHT):
                    gate = ALU.min if t < 3 else ALU.max
                    r = s_pool.tile([128, BC], F16, tag="r", name=f"r{ch}_{t}")
                    nc.vector.tensor_scalar(r[:], v, float(cj) - 0.5, 0.0,
                                            ALU.subtract, gate)
                    nc.scalar.activation(plane(2 + t), r[:], AF.Square)

            def mm_chunk(ft, ch):
                """8 o-tile psum banks, 9-plane accumulation; features move."""
                otc = o_pool.tile([128, N_OT * BC], F16, tag="otc",
                                  name=f"otc{ch}")
                for ot in range(N_OT):
                    ps = ps_pool.tile([128, BC], F32, tag=f"ps{ot}",
                                      name=f"ps{ch}_{ot}")
                    for j in range(N_PLANES):
                        nc.tensor.matmul(
                            ps[:],
                            wt[:, j * O + ot * 128:j * O + (ot + 1) * 128],
                            ft[:, j * BC:(j + 1) * BC],
                            start=(j == 0), stop=(j == N_PLANES - 1),
                        )
                    dst = otc[:, ot * BC:(ot + 1) * BC]
                    if ot % 2 == 0:
                        nc.vector.tensor_copy(dst, ps[:])
                    else:
                        nc.scalar.copy(dst, ps[:])
                eng = nc.sync if ch % 2 == 0 else nc.scalar
                eng.dma_start(
                    yT_d[:, ch * BC:(ch + 1) * BC].rearrange(
                        "(t p) b -> p t b", p=128),
                    otc[:].rearrange("p (t b) -> p t b", t=N_OT))

            def body(_iv=None):
                xt = state['xt']
                nxt = load_xt()          # next body's x, DMA overlaps this body
                for ch in range(NCH):
                    ft_cur = state['ft']
                    # produce the NEXT chunk's features (chunk 0 of the next
                    # body when ch==3) while this chunk's matmuls run
                    ft_next = f_pool.tile([128, N_PLANES * BC], F16,
                                          tag="ft", name=f"ft{(ch + 1) % 2}")
                    if ch < NCH - 1:
                        produce(ft_next, xt[:, (ch + 1) * BC:(ch + 2) * BC], ch + 1)
                    else:
                        produce(ft_next, nxt[:, 0:BC], 0)
                    mm_chunk(ft_cur, ch)
                    state['ft'] = ft_next
                state['xt'] = nxt

            def head():
                xt0 = x_pool.tile([128, B], F32, tag="xt", name="xt_h")
                nc.sync.dma_start(xt0[:, 0:BC], xT_d[:, 0:BC])
                nc.scalar.dma_start(xt0[:, BC:], xT_d[:, BC:])
                ft0 = f_pool.tile([128, N_PLANES * BC], F16, tag="ft",
                                  name="ft_h")
                produce(ft0, xt0[:, 0:BC], 0)
                state.update(xt=xt0, ft=ft0)

            head()
            if unroll > 1:
                assert reps == 1
                for _ in range(unroll):
                    body()
            elif reps == 1:
                body()
            else:
                assert reps % loop_unroll == 0
                with tc.For_i(0, reps // loop_unroll, 1,
                              hint_engines=(list(mybir.ALL_ENGINES)
                                            if hint else ()),
                              staggered_reset=staggered) as iv:
                    for _ in range(loop_unroll):
                        body(iv)
    nc.compile()
    return nc


_cached_nc = None


def _get_nc():
    global _cached_nc
    if _cached_nc is None:
        _cached_nc = build_kernel(reps=1)
    return _cached_nc


def prepare_inputs(x, spline_parameters, w_b, w_s):
    """Host-side prep: returns (in_maps, bias, w_b, silu_sum) for the 8 cores."""
    x = np.ascontiguousarray(np.asarray(x, np.float32))
    P = np.asarray(spline_parameters, np.float32)
    w_b = float(np.asarray(w_b))
    W, bias = fold_weights(P, float(np.asarray(w_s)))
    xd = x.astype(np.float64)
    silu_sum = (xd / (1.0 + np.exp(-xd))).sum(axis=1)          # (B,)
    xT = np.ascontiguousarray(x.T)                             # (I, B)
    in_maps = []
    for c in range(N_CORES):
        sl = slice(c * I_LOC, (c + 1) * I_LOC)
        in_maps.append({
            "xT": np.ascontiguousarray(xT[sl, :]),
            "Wf": np.ascontiguousarray(
                W[:, sl, :].reshape(N_PLANES * I_LOC, O)),
        })
    return in_maps, bias, w_b, silu_sum


def kernel(x, spline_parameters, w_b, w_s):
    in_maps, bias, w_b, silu_sum = prepare_inputs(x, spline_parameters, w_b, w_s)
    nc = _get_nc()
    try:
        res = run_bass_kernel_spmd(nc, in_maps, core_ids=list(range(N_CORES)))
    except Exception:
        # transient device flakes (NRT_EXEC_UNIT_UNRECOVERABLE) happen; retry
        res = run_bass_kernel_spmd(nc, in_maps, core_ids=list(range(N_CORES)))
    acc = np.zeros((O, B), np.float64)
    for c in range(N_CORES):
        acc += res.results[c]["yT"].astype(np.float64)
    out = acc.T + bias[None, :]
    out += (w_b * silu_sum)[:, None]
    return out.astype(np.float32)
